# revision 48
# baseline (speedup 1.0000x reference)
"""Trainium2 Bass kernel for masked multi-head attention.

Problem: B=4, N=2048, D=1024, H=16 heads (DK=64).
  q = x @ Wq.T + bq ; k = x @ Wk.T + bk ; v = x @ Wv.T + bv
  scores = q k^T / sqrt(DK), masked (mask==0 -> -1e9), softmax, z = w v

Sharding: 8 cores = 4 batches x 2 head-groups (8 heads each). Each core
gets its batch's x (transposed), its head-group's weight slices
(transposed) and the batch mask (transposed, cast to bf16). Each core
computes z^T for its 8 heads; the host transposes/concats the results.

Device-side layout (all "transposed", i.e. feature/key dim on partitions):
  Q^T, K^T : [512, 2048]  (head-dim on partitions, 64 per head)
  V'       : per key-chunk [128, 8 heads, 65] = V columns + ones column
  S^T      : [128 keys, 2 heads, 512 queries] in PSUM -- the two heads of a
             pair computed by row-packed K=64 matmuls (head A weights on
             array rows 0-63, head B on 64-127) so the full PE array is
             active (keeps the HAM clock-gate at 2.4 GHz).
  P = exp(S^T/8) -> bf16 (scores bounded ~|2|, no max subtraction needed)
  PM = P * maskT (bf16, DVE 2x mode)
  Z'^T[65, 512] += V'[128,65].T @ PM  (row 64 = softmax denominators)
  z^T = Z'^T[0:64] * recip(broadcast(Z'^T[64]))
Z-matmuls are emitted one iteration behind the S-matmuls so the PE's
in-order queue never waits on the exp/mask chain.
"""

import os
import sys
from contextlib import ExitStack

import numpy as np

for _p in ("/opt/trn_rl_repo", "/root/.axon_site/_ro/trn_rl_repo"):
    if os.path.isdir(_p) and _p not in sys.path:
        sys.path.append(_p)

import ml_dtypes

import concourse.bass as bass
import concourse.tile as tile
from concourse import bacc, mybir
from concourse.bass_utils import run_bass_kernel_spmd

B, N, D, H = 4, 2048, 1024, 16
DK = D // H          # 64
HPC = 8              # heads per core
DC = HPC * DK        # 512, per-core model dim
NCORES = 8
BF16 = ml_dtypes.bfloat16

f32 = mybir.dt.float32
bf16 = mybir.dt.bfloat16
AF = mybir.ActivationFunctionType


def build_bass():
    nc = bacc.Bacc(None, target_bir_lowering=False)

    xT = nc.dram_tensor("xT", [D, N], bf16, kind="ExternalInput")
    wq = nc.dram_tensor("wq", [D, DC], bf16, kind="ExternalInput")
    wk = nc.dram_tensor("wk", [D, DC], bf16, kind="ExternalInput")
    wv = nc.dram_tensor("wv", [D, DC], bf16, kind="ExternalInput")
    bq2 = nc.dram_tensor("bq2", [128, 4], f32, kind="ExternalInput")
    bk2 = nc.dram_tensor("bk2", [128, 4], f32, kind="ExternalInput")
    bvrow = nc.dram_tensor("bvrow", [1, DC], bf16, kind="ExternalInput")
    maskT = nc.dram_tensor("maskT", [N, N], bf16, kind="ExternalInput")
    zT = nc.dram_tensor("zT", [HPC, DK, N], f32, kind="ExternalOutput")

    with tile.TileContext(nc) as tc, ExitStack() as ctx:
        persist = ctx.enter_context(tc.tile_pool(name="persist", bufs=1))
        dram = ctx.enter_context(tc.tile_pool(name="dram", bufs=2, space="DRAM"))

        mask_sb = persist.tile([128, 16, N], bf16)
        QT_sb = persist.tile([128, 4, N], bf16)
        KT_sb = persist.tile([128, 4, N], bf16)
        V_sb = persist.tile([128, 16, HPC, DK + 1], bf16)
        bq_sb = persist.tile([128, 4], f32)
        bk_sb = persist.tile([128, 4], f32)
        bv_sb = persist.tile([1, DC], bf16)
        ones_sb = persist.tile([1, 128], bf16)
        xT_sb = persist.tile([128, 8, N], bf16)
        wq_sb = persist.tile([128, 8, DC], bf16)
        wk_sb = persist.tile([128, 8, DC], bf16)
        wv_sb = persist.tile([128, 8, DC], bf16)

        nc.vector.memset(ones_sb, 1.0)
        nc.vector.memset(V_sb[:, :, :, DK : DK + 1], 1.0)
        xTr = xT.rearrange("(c p) n -> p c n", p=128)
        wvr = wv.rearrange("(c p) m -> p c m", p=128)
        nc.sync.dma_start(out=wv_sb[:, 0:2, :], in_=wvr[:, 0:2, :])
        nc.sync.dma_start(
            out=xT_sb[:, :, 0:512], in_=xTr[:, :, 0:512]
        )
        for ws in range(1, 4):
            nc.sync.dma_start(
                out=wv_sb[:, 2 * ws : 2 * ws + 2, :], in_=wvr[:, 2 * ws : 2 * ws + 2, :]
            )
        for xq in range(1, 4):
            nc.sync.dma_start(
                out=xT_sb[:, :, xq * 512 : (xq + 1) * 512],
                in_=xTr[:, :, xq * 512 : (xq + 1) * 512],
            )
        nc.sync.dma_start(out=bv_sb, in_=bvrow[:, :])
        nc.sync.dma_start(out=wq_sb, in_=wq.rearrange("(c p) m -> p c m", p=128))
        nc.sync.dma_start(out=wk_sb, in_=wk.rearrange("(c p) m -> p c m", p=128))
        nc.sync.dma_start(out=bq_sb, in_=bq2[:, :])
        nc.sync.dma_start(out=bk_sb, in_=bk2[:, :])
        mT = maskT.rearrange("(m p) n -> p m n", p=128)
        for mq in range(4):
            nc.sync.dma_start(
                out=mask_sb[:, mq * 4 : (mq + 1) * 4, :],
                in_=mT[:, mq * 4 : (mq + 1) * 4, :],
            )

        def mm_one(out, lhsT, rhs, start, stop):
            nc.tensor.matmul(out, lhsT=lhsT, rhs=rhs, start=start, stop=stop)

        with tc.tile_pool(name="qkvps", bufs=4, space="PSUM") as qkvps:
            # V first (needs only xT + wv loaded) so the PE has work while
            # the mask/Q/K weights are still streaming in and ACT has nothing
            # to do anyway.  V natural: out[n, d] = x^T.T @ Wv^T + bv
            for mch in range(16):
                ps = qkvps.tile([128, 512], f32, tag="ps")
                for k in range(8):
                    mm_one(
                        ps,
                        xT_sb[:, k, mch * 128 : (mch + 1) * 128],
                        wv_sb[:, k, :],
                        start=(k == 0),
                        stop=False,
                    )
                nc.tensor.matmul(
                    ps, lhsT=ones_sb, rhs=bv_sb, start=False, stop=True
                )
                nc.vector.tensor_copy(
                    V_sb[:, mch, :, 0:DK],
                    ps.rearrange("p (h d) -> p h d", h=HPC),
                )
            # Q^T and K^T: out[d, n] = sum_k W^T[k, d] * x^T[k, n]; bias is
            # folded into the PSUM->SBUF copy on ACT (per-partition bias).
            # d-chunk-major so head pair 0's Q/K finish first and attention
            # can start while the rest of QKV still runs.
            for dch in range(4):
                for w_sb, b_sb, dst in ((wq_sb, bq_sb, QT_sb), (wk_sb, bk_sb, KT_sb)):
                    for nch in range(4):
                        ps = qkvps.tile([128, 512], f32, tag="ps")
                        for k in range(8):
                            mm_one(
                                ps,
                                w_sb[:, k, dch * 128 : (dch + 1) * 128],
                                xT_sb[:, k, nch * 512 : (nch + 1) * 512],
                                start=(k == 0),
                                stop=(k == 7),
                            )
                        nc.scalar.activation(
                            dst[:, dch, nch * 512 : (nch + 1) * 512],
                            ps,
                            AF.Identity,
                            bias=b_sb[:, dch : dch + 1],
                            scale=1.0,
                        )

        # Attention over head pairs; queries in 512-wide quarters.
        with (
            tc.tile_pool(name="spool", bufs=2, space="PSUM") as spool,
            tc.tile_pool(name="zpool", bufs=4, space="PSUM") as zpool,
            tc.tile_pool(name="pp", bufs=3) as pp,
            tc.tile_pool(name="pmp", bufs=3) as pmp,
            tc.tile_pool(name="np_", bufs=2) as np_,
            tc.tile_pool(name="znp", bufs=2) as znp,
        ):
            blocks = [(hp, nq) for hp in range(HPC // 2) for nq in range(4)]
            zaccs = {}
            pending = []        # (block_idx, zmms_fn) carried across blocks
            norm_after = {}     # block_idx -> norm emitter

            def emit_norm(bi):
                hp, nq = blocks[bi]
                q0 = nq * 512
                Za, Zb = zaccs.pop(bi)
                # normalization: z = Z[0:64] * recip(broadcast(Z[64]))
                srow = np_.tile([33, 512], f32, tag="srow")
                nc.vector.tensor_copy(srow[0:1, :], Za[DK : DK + 1, :])
                nc.vector.tensor_copy(srow[32:33, :], Zb[DK : DK + 1, :])
                sd = dram.tile([2, 512], f32, tag="sd")
                nc.sync.dma_start(out=sd[0:1, :], in_=srow[0:1, :])
                nc.sync.dma_start(out=sd[1:2, :], in_=srow[32:33, :])
                sbc = np_.tile([128, 512], f32, tag="sbc")
                nc.sync.dma_start(
                    out=sbc[0:DK, :], in_=sd[0:1, :].partition_broadcast(DK)
                )
                nc.sync.dma_start(
                    out=sbc[DK:128, :], in_=sd[1:2, :].partition_broadcast(DK)
                )
                rbc = np_.tile([128, 512], f32, tag="rbc")
                nc.vector.reciprocal_approx_fast(rbc, sbc)
                zn = znp.tile([128, 512], f32, tag="zn")
                nc.vector.tensor_mul(zn[0:DK, :], Za[0:DK, :], rbc[0:DK, :])
                nc.vector.tensor_mul(zn[DK:128, :], Zb[0:DK, :], rbc[DK:128, :])
                nc.sync.dma_start(
                    out=zT[2 * hp, :, q0 : q0 + 512], in_=zn[0:DK, :]
                )
                nc.sync.dma_start(
                    out=zT[2 * hp + 1, :, q0 : q0 + 512], in_=zn[DK:128, :]
                )

            for bi, (hp, nq) in enumerate(blocks):
                dch = hp
                q0 = nq * 512
                Za = zpool.tile([DK + 1, 512], f32, tag="z")
                Zb = zpool.tile([DK + 1, 512], f32, tag="z")
                zaccs[bi] = (Za, Zb)
                Zacc = (Za, Zb)
                for m in range(16):
                    S = spool.tile([128, 2, 512], f32, tag="s")
                    for j in range(2):
                        off = j * DK
                        nc.tensor.matmul(
                            S[:, j, :],
                            lhsT=KT_sb[
                                off : off + DK, dch, m * 128 : (m + 1) * 128
                            ],
                            rhs=QT_sb[off : off + DK, dch, q0 : q0 + 512],
                            start=True,
                            stop=True,
                        )
                    # Z-matmuls lag one iteration behind (carried across
                    # block boundaries so the PE never drains at a boundary);
                    # once a block's last Z fires, its normalization follows.
                    if pending:
                        pbi, fn = pending.pop(0)
                        fn()
                        if pbi != bi:
                            emit_norm(pbi)
                    P = pp.tile([128, 2, 512], bf16, tag="p")
                    nc.scalar.activation(P, S, AF.Exp, scale=1.0 / np.sqrt(DK))
                    PM = pmp.tile([128, 2, 512], bf16, tag="pm")
                    for j in range(2):
                        nc.vector.tensor_mul(
                            PM[:, j, :], P[:, j, :], mask_sb[:, m, q0 : q0 + 512]
                        )

                    def zmms(m=m, PM=PM, Zacc=Zacc, hp=hp):
                        for j in range(2):
                            nc.tensor.matmul(
                                Zacc[j],
                                lhsT=V_sb[:, m, 2 * hp + j, :],
                                rhs=PM[:, j, :],
                                start=(m == 0),
                                stop=(m == 15),
                            )

                    pending.append((bi, zmms))
            for pbi, fn in pending:
                fn()
                emit_norm(pbi)

    return nc


def host_prep(x, x_mask, direction, Wq, bq, Wk, bk, Wv, bv):
    """Shard + lay out inputs for the 8 cores. Core c: batch c%4, head-group c//4."""
    x = np.asarray(x, dtype=np.float32)
    x_mask = np.asarray(x_mask)
    direction = int(np.asarray(direction))
    in_maps = []
    for c in range(NCORES):
        b, g = c % 4, c // 4
        rows = slice(g * DC, (g + 1) * DC)
        m = x_mask[b]
        if direction != 0:
            m = m.T
        in_maps.append(
            {
                "xT": np.ascontiguousarray(x[b].T).astype(BF16),
                "wq": np.ascontiguousarray(np.asarray(Wq)[rows].T).astype(BF16),
                "wk": np.ascontiguousarray(np.asarray(Wk)[rows].T).astype(BF16),
                "wv": np.ascontiguousarray(np.asarray(Wv)[rows].T).astype(BF16),
                "bq2": np.ascontiguousarray(
                    np.asarray(bq, dtype=np.float32)[rows].reshape(4, 128).T
                ),
                "bk2": np.ascontiguousarray(
                    np.asarray(bk, dtype=np.float32)[rows].reshape(4, 128).T
                ),
                "bvrow": np.asarray(bv, dtype=np.float32)[rows]
                .reshape(1, DC)
                .astype(BF16),
                "maskT": np.ascontiguousarray(m).astype(BF16),
            }
        )
    return in_maps


def assemble(results):
    """results: per-core dict with 'zT' [8, 64, 2048] -> full z [B, N, D]."""
    z = np.empty((B, N, D), dtype=np.float32)
    for c in range(NCORES):
        b, g = c % 4, c // 4
        zt = np.asarray(results[c]["zT"], dtype=np.float32)  # [8, 64, N]
        z[b, :, g * DC : (g + 1) * DC] = zt.transpose(2, 0, 1).reshape(N, DC)
    return z


def _ensure_device_backend():
    """Make sure jax's default backend exposes the 8 NeuronCores (the host
    may have flipped jax_platforms to cpu to run the reference)."""
    import jax

    try:
        devs = jax.devices()
    except Exception:
        devs = []
    if len([d for d in devs if d.platform != "cpu"]) < NCORES:
        jax.config.update("jax_platforms", "axon")


def run(inputs, trace=False, tmpdir=None):
    _ensure_device_backend()
    nc = build_bass()
    nc.finalize()
    in_maps = host_prep(**inputs)
    res = run_bass_kernel_spmd(
        nc,
        in_maps,
        core_ids=list(range(NCORES)),
        trace=trace,
        tmpdir=tmpdir,
    )
    return assemble(res.results), res


def kernel(**inputs) -> np.ndarray:
    out, _ = run(inputs)
    return out
